# revision 24
# baseline (speedup 1.0000x reference)
"""Single-head attention (B=8, T=2048, C=512, d_k=64) on 8 Trainium2 cores.

Data-parallel over batch B - one batch element per NeuronCore, no collectives.

v6 design (v1 fp32: 208us, v2 f32r: 91us, v4 bf16: 91us):
  - All matmuls bf16 (1 PE cycle/row; fp32 is 2-pass = 4cyc/row). fp32
    accumulation in PSUM; rel err ~4e-3 vs the 2e-2 gate.
  - x tiles: DMA (GpSimd queue) -> DVE cast bf16 -> 4 PE transposes in bf16
    (1cyc/row, writing a bf16-bitcast view of the fp32 PSUM work slot) ->
    one strided DVE copy into bf16 x^T.
  - Q^T/K^T duplicated on both partition halves by doubling Wq/Wk along the
    stationary free dim (one matmul writes both copies), enabling 2-way
    row-packed S^T matmuls (tile_position rows 0-63 / 64-127, concurrent).
  - V via V^T projections (N=512) + 16 small PE transposes; ones column
    appended so the softmax denominator falls out of the AV accumulation.
  - ScalarE's serial exp chain (~35us for 4.2M elements) is the floor.
    KEY SCHEDULING FACT (measured v4): the Tile scheduler runs each engine
    in emission-priority order, so anything emitted before the main loop
    executes before the first S matmul. v6 therefore emits only the minimal
    critical path (4 tile loads + projection chunk 0) before the main loop,
    orders steps by input availability, and spreads ALL remaining work
    (loads, projections, V transposes, the first half's epilogue) as
    fine-grained fillers between main-loop steps.
  - A short PE warmup spinner of regular matmuls is interleaved with the
    first tile loads so the HAM clock-gate reaches 8/8 before the
    projection stream (transpose-mode ops don't count as PE activity).
"""

import numpy as np
from contextlib import ExitStack

import concourse.bass as bass
import concourse.tile as tile
from concourse import bacc
from concourse import mybir
from concourse.bass_utils import run_bass_kernel_spmd
from concourse.masks import make_identity

B, T, C, DK = 8, 2048, 512, 64
N_CORES = 8
FP32 = mybir.dt.float32
BF16 = mybir.dt.bfloat16
P = 128
TT = T // P      # 16 token tiles
CCH = C // P     # 4 contraction chunks
NB = 512         # PSUM-bank-limited matmul output free dim
SCALE = 1.0 / np.sqrt(np.float32(DK))

_cached = {}


def _build_nc():
    nc = bacc.Bacc("TRN2", target_bir_lowering=False, debug=False)
    x_d = nc.declare_dram_parameter("x", [T, C], FP32, isOutput=False)
    wq_d = nc.declare_dram_parameter("Wq", [C, DK], FP32, isOutput=False)
    wk_d = nc.declare_dram_parameter("Wk", [C, DK], FP32, isOutput=False)
    wv_d = nc.declare_dram_parameter("Wv", [C, DK], FP32, isOutput=False)
    out_d = nc.declare_dram_parameter("out", [T, DK], FP32, isOutput=True)

    x_t = x_d.rearrange("(tt p) c -> tt p c", p=P)          # [16,128,512]
    out_t = out_d.rearrange("(tt p) d -> tt p d", p=P)      # [16,128,64]

    with ExitStack() as ctx:
        tc = ctx.enter_context(tile.TileContext(nc))
        const = ctx.enter_context(tc.tile_pool(name="const", bufs=1))
        xload = ctx.enter_context(tc.tile_pool(name="xload", bufs=6))
        ppool = ctx.enter_context(tc.tile_pool(name="ppool", bufs=4))
        outp = ctx.enter_context(tc.tile_pool(name="outp", bufs=4))
        spool = ctx.enter_context(tc.tile_pool(name="spool", bufs=2, space="PSUM"))
        opool = ctx.enter_context(tc.tile_pool(name="opool", bufs=1, space="PSUM"))
        wpool = ctx.enter_context(tc.tile_pool(name="wpool", bufs=2, space="PSUM"))

        identity = const.tile([P, P], FP32)
        make_identity(nc, identity)
        id16 = const.tile([P, P], BF16)
        nc.vector.tensor_copy(out=id16, in_=identity)

        # warm the exp table set early (~2.7us ACT_TABLE_LOAD)
        dum_i = const.tile([P, 1], FP32, name="dumi")
        dum_o = const.tile([P, 1], FP32, name="dumo")
        nc.vector.memset(dum_i, 0.0)
        nc.scalar.activation(out=dum_o, in_=dum_i,
                             func=mybir.ActivationFunctionType.Exp)
        nc.vector.tensor_copy(out=dum_i, in_=dum_o)

        xT = const.tile([P, CCH, T], BF16)      # x^T chunks, bf16

        def tile_load(tt):
            x_tile = xload.tile([P, C], FP32, tag="x_tile")
            # spread the first tiles across all three DMA-issuing queues so
            # they land in parallel right after the preamble
            eng = (nc.gpsimd, nc.sync, nc.scalar)[tt % 3 if tt < 3 else 0]
            eng.dma_start(out=x_tile, in_=x_t[tt])
            tps = wpool.tile([P, NB], FP32, tag="wps", name="tps")
            for ch in range(CCH):
                sl = slice(ch * P, (ch + 1) * P)
                if tt < 2:
                    # regular matmul x_chunk.T @ I: same result, but COUNTS
                    # as PE activity for the HAM clock-gate - transposes
                    # don't, so a transpose-only load phase runs at 1.2GHz.
                    # Two tiles (~16 matmul passes) warm the clock; the
                    # continuous proj/S/AV stream then keeps it at 8/8.
                    nc.tensor.matmul(tps[:, sl], lhsT=x_tile[:, sl],
                                     rhs=identity, start=True, stop=True)
                else:
                    nc.tensor.transpose(tps[:, sl], x_tile[:, sl], identity)
            nc.vector.tensor_copy(
                out=xT[:, :, tt * P:(tt + 1) * P],
                in_=tps[:, :].rearrange("p (ch t) -> p ch t", ch=CCH))

        def load_weights():
            wq_s = const.tile([P, CCH, DK], FP32, name="wqs")
            wk_s = const.tile([P, CCH, DK], FP32, name="wks")
            wv_s = const.tile([P, CCH, DK], FP32, name="wvs")
            nc.sync.dma_start(out=wq_s, in_=wq_d.rearrange("(ch p) d -> p ch d", p=P))
            nc.sync.dma_start(out=wk_s, in_=wk_d.rearrange("(ch p) d -> p ch d", p=P))
            nc.sync.dma_start(out=wv_s, in_=wv_d.rearrange("(ch p) d -> p ch d", p=P))
            # bf16 weights; wq/wk doubled along the stationary free dim so one
            # matmul emits Q^T/K^T on BOTH partition halves of the output
            wq2 = const.tile([P, CCH, P], BF16, name="wq2")
            wk2 = const.tile([P, CCH, P], BF16, name="wk2")
            wv16 = const.tile([P, CCH, DK], BF16, name="wv16")
            nc.vector.tensor_copy(out=wq2[:, :, 0:DK], in_=wq_s)
            nc.vector.tensor_copy(out=wq2[:, :, DK:P], in_=wq_s)
            nc.vector.tensor_copy(out=wk2[:, :, 0:DK], in_=wk_s)
            nc.vector.tensor_copy(out=wk2[:, :, DK:P], in_=wk_s)
            nc.vector.tensor_copy(out=wv16, in_=wv_s)
            return wq2, wk2, wv16

        qT2 = const.tile([P, T], BF16)          # Q^T dup on both halves
        kT2 = const.tile([P, T], BF16)          # K^T dup on both halves
        vTs = const.tile([DK, T], FP32)         # V^T
        v_s = const.tile([P, TT, DK + 1], BF16)  # V with ones col
        ones = const.tile([P, TT], FP32, name="ones")
        nc.vector.memset(ones, 1.0)
        nc.vector.tensor_copy(out=v_s[:, :, DK], in_=ones)
        oT = const.tile([DK + 1, T], BF16)      # out^T staging

        def proj_q(ic):
            sl = slice(ic * NB, (ic + 1) * NB)
            pq = wpool.tile([P, NB], FP32, tag="wps", name="pq")
            for ch in range(CCH):
                nc.tensor.matmul(pq, lhsT=wq2[:, ch, :], rhs=xT[:, ch, sl],
                                 start=(ch == 0), stop=(ch == CCH - 1))
            nc.vector.tensor_copy(out=qT2[:, sl], in_=pq)

        def proj_k(ic):
            sl = slice(ic * NB, (ic + 1) * NB)
            pk = wpool.tile([P, NB], FP32, tag="wps", name="pk")
            for ch in range(CCH):
                nc.tensor.matmul(pk, lhsT=wk2[:, ch, :], rhs=xT[:, ch, sl],
                                 start=(ch == 0), stop=(ch == CCH - 1))
            nc.vector.tensor_copy(out=kT2[:, sl], in_=pk)

        def proj_v(ic):
            sl = slice(ic * NB, (ic + 1) * NB)
            pv = wpool.tile([P, NB], FP32, tag="wps", name="pv")
            for ch in range(CCH):
                nc.tensor.matmul(pv[0:DK, :], lhsT=wv16[:, ch, :],
                                 rhs=xT[:, ch, sl],
                                 start=(ch == 0), stop=(ch == CCH - 1))
            nc.vector.tensor_copy(out=vTs[:, sl], in_=pv[0:DK, :])

        def vtrans(j):
            vps = wpool.tile([P, NB], FP32, tag="wps", name="vps")
            nc.tensor.transpose(
                vps[:, 0:DK], vTs[:, j * P:(j + 1) * P], identity[0:DK, 0:DK])
            nc.vector.tensor_copy(out=v_s[:, j, 0:DK], in_=vps[:, 0:DK])

        # ---- main loop: software-pipelined S -> exp -> AV over 32 steps ----
        # step = (half, jj, qc): key pair (2jj, 2jj+1) x query 512-chunk.
        # h0 ordered by input availability: the first steps only need
        # projection chunk 0 (x tiles 0-3).
        order_h0 = [(0, 0), (1, 0), (0, 1), (1, 1), (2, 0), (2, 1), (3, 0),
                    (3, 1), (4, 0), (4, 1), (5, 0), (5, 1), (6, 0), (6, 1),
                    (7, 0), (7, 1)]
        # h1 qc-major: query chunk 2 finishes 8 steps before chunk 3, so
        # its epilogue tiles overlap the last steps instead of trailing
        steps = [(0, jj, qc) for jj, qc in order_h0] + \
                [(1, jj, 0) for jj in range(8)] + \
                [(1, jj, 1) for jj in range(8)]

        def emit_S(h, jj, qc):
            s = spool.tile([P, 2 * NB], FP32, tag="sps")
            q0 = h * 1024 + qc * NB
            ja = slice(2 * jj * P, (2 * jj + 1) * P)
            jb = slice((2 * jj + 1) * P, (2 * jj + 2) * P)
            nc.tensor.matmul(s[:, 0:NB], lhsT=kT2[0:DK, ja],
                             rhs=qT2[0:DK, q0:q0 + NB],
                             start=True, stop=True)
            nc.tensor.matmul(s[:, NB:2 * NB], lhsT=kT2[DK:P, jb],
                             rhs=qT2[DK:P, q0:q0 + NB],
                             start=True, stop=True)
            return s

        o_ps = {}

        def emit_tail(i):
            h, jj, qc = steps[i]
            if jj == 0 and qc == 0:
                o_ps[h] = opool.tile([DK + 1, 2 * NB], FP32, tag="ops",
                                     name=f"ops{h}")
            pT = ppool.tile([P, 2 * NB], BF16, tag="pT")
            nc.scalar.activation(out=pT, in_=s_tiles[i],
                                 func=mybir.ActivationFunctionType.Exp,
                                 scale=float(SCALE))
            osl = o_ps[h][:, qc * NB:(qc + 1) * NB]
            nc.tensor.matmul(osl, lhsT=v_s[:, 2 * jj, :],
                             rhs=pT[:, 0:NB],
                             start=(jj == 0), stop=False, skip_group_check=True)
            nc.tensor.matmul(osl, lhsT=v_s[:, 2 * jj + 1, :],
                             rhs=pT[:, NB:2 * NB],
                             start=False, stop=(jj == TT // 2 - 1),
                             skip_group_check=True)
            if jj == TT // 2 - 1:
                q0 = h * 1024 + qc * NB
                nc.vector.tensor_copy(
                    out=oT[:, q0:q0 + NB],
                    in_=o_ps[h][:, qc * NB:(qc + 1) * NB])

        def epilogue(tt):
            eps = wpool.tile([P, NB], FP32, tag="wps", name="eps")
            e16 = eps[:, :].bitcast(BF16)
            nc.tensor.transpose(
                e16[:, 0:DK + 1], oT[:, tt * P:(tt + 1) * P],
                id16[0:DK + 1, 0:DK + 1])
            rc = outp.tile([P, 1], FP32, tag="rc", bufs=2)
            nc.vector.reciprocal(rc, e16[:, DK:DK + 1])
            ot = outp.tile([P, DK], FP32, tag="ot")
            nc.vector.tensor_scalar_mul(ot, e16[:, 0:DK], rc)
            nc.sync.dma_start(out=out_t[tt], in_=ot)

        # ---- interleaved emission: minimal critical path first ----
        tile_load(0)
        tile_load(1)
        tile_load(2)
        tile_load(3)
        wq2, wk2, wv16 = load_weights()
        proj_q(0)
        proj_k(0)

        # S(0) and S(1) emitted adjacently so the exp chain starts with no
        # PE-FIFO gap; proj_v/vtrans follow - v_s[0,1] is only needed by
        # AV(0), one exp (~1.1us) later
        s_tiles = {}
        s_tiles[0] = emit_S(*steps[0])
        s_tiles[1] = emit_S(*steps[1])
        # critical path to S(2): proj_q(1) needs x tiles 4-7, ALL of which
        # must be emitted before it (later-emitted writes order after reads)
        tile_load(4)
        tile_load(5)
        tile_load(6)
        tile_load(7)
        proj_q(1)
        s_tiles[2] = emit_S(*steps[2])
        proj_v(0)
        vtrans(0)
        vtrans(1)

        # fillers[k] are emitted just before emit_S(steps[k]) (the S for
        # step k is emitted one iteration early for software pipelining).
        # Every vtrans(j) must be emitted at or before the step whose AV
        # reads v_s[j], and every proj before the S/vtrans that reads it.
        fillers = {
            1: lambda: [vtrans(2), vtrans(3)],
            2: lambda: [proj_k(1), proj_v(1)],
            3: lambda: [vtrans(4), vtrans(5), tile_load(8)],
            4: lambda: [vtrans(6), vtrans(7), tile_load(9)],
            5: lambda: [tile_load(10), tile_load(11), proj_q(2)],
            6: lambda: [proj_k(2), proj_v(2)],
            7: lambda: [vtrans(8), vtrans(9), tile_load(12)],
            8: lambda: [vtrans(10), vtrans(11), tile_load(13)],
            9: lambda: [tile_load(14), tile_load(15), proj_q(3)],
            10: lambda: [proj_k(3), proj_v(3)],
            11: lambda: [vtrans(12), vtrans(13)],
            12: lambda: [vtrans(14), vtrans(15)],
            17: lambda: [epilogue(0), epilogue(1)],
            18: lambda: [epilogue(2), epilogue(3)],
            19: lambda: [epilogue(4), epilogue(5)],
            20: lambda: [epilogue(6), epilogue(7)],
            25: lambda: [epilogue(8), epilogue(9)],
            26: lambda: [epilogue(10), epilogue(11)],
        }

        for i in range(len(steps)):
            if i + 1 in fillers:
                fillers[i + 1]()
            if i + 1 < len(steps) and i + 1 not in s_tiles:
                s_tiles[i + 1] = emit_S(*steps[i + 1])
            emit_tail(i)
            del s_tiles[i]

        for tt in range(12, TT):
            epilogue(tt)

    nc.compile()
    return nc


def _get_nc():
    if "nc" not in _cached:
        _cached["nc"] = _build_nc()
    return _cached["nc"]


def kernel(x, Wq, Wk, Wv, **run_kwargs):
    x = np.asarray(x, dtype=np.float32)
    Wq = np.asarray(Wq, dtype=np.float32)
    Wk = np.asarray(Wk, dtype=np.float32)
    Wv = np.asarray(Wv, dtype=np.float32)
    nc = _get_nc()
    in_maps = [
        {"x": np.ascontiguousarray(x[b]), "Wq": Wq, "Wk": Wk, "Wv": Wv}
        for b in range(B)
    ]
    res = run_bass_kernel_spmd(nc, in_maps, list(range(N_CORES)), **run_kwargs)
    out = np.stack([res.results[b]["out"] for b in range(B)], axis=0)
    if run_kwargs:
        _cached["last_result"] = res
    return out
